# revision 43
# baseline (speedup 1.0000x reference)
"""Trainium2 Bass kernel for nn_BilinearAttnPool (B=32, C=2048, H=24, W=12, M=8).

Math (exactness argument):
  reference: attn = relu(BN(conv1x1(f)))  (attn >= 0)
             x = clip(f * attn, min=1e-6) ** 3 ; pooled = mean_hw(x) ** (1/3)
  Since attn >= 0:  clip(f*attn, eps)^3 = attn^3 * relu(f)^3  up to eps^3=1e-18
  terms (negligible).  So pooled(b,m,c)^3 ~ sum_hw attn^3(b,m,hw) relu(f)^3(c,hw)
  -- a matmul over hw.  The 1/HW mean and any global scale cancel in the final
  L2 normalize, which (with the sign-sqrt; pooled >= 0 => sqrt) is done on the
  host from the device's z = s^(1/6) output.

Design (v10):
  - Dual feature upload, both fp8 e3m4 (4-bit mantissa): fnat (c-on-partitions,
    conv contraction operand, signed f) and ftq = relu(f)^1.5 (hw-on-
    partitions).  The ^1.5 encoding is range compression: e3m4's normal range
    covers [0.016, 15.5] and x^1.5 <= 12.9, so the on-device square (the
    pooling operand relu(f)^3) sees only ~2x the quantization error --
    uploading x^3 directly would overflow/underflow e3m4 and fail the gate.
    End-to-end rel-err ~9.8e-3 vs the 2e-2 gate.
  - The 4 samples' hw axes are CONCATENATED: 4*288 = 1152 = 9 exact chunks of
    128 partitions (zero padding waste).  Squares split DVE/ACT per chunk.
  - conv lhsT w2rep has the (BN-folded) weights replicated in all four 8-col
    slots, so conv(b) writes rows j=8b+m (the other bands hold junk
    duplicates).  The attn relu reads all 32 rows with a per-sample bias
    column that is -1e9 outside band b: junk rows become exact zeros, giving
    attn/a3 a block-diagonal [32, 1152] "global hw" layout.
  - 9 PE transposes (128-col slabs of a3) build a3t[p, k, 8b+m]; one matmul
    per (chunk, 512-col block) then pools all 4 samples at once (cross-sample
    terms get zero weight).  The 4 col-blocks run concurrently in the PE's
    col-tiled 32x32 sub-arrays.
  - Pool output: ONE psum bank [128, 512]; quadrant q rows 32q+8b+m hold
    c in [512q, 512q+512).  z = exp(ln(s)/6) -> fp16, DMA out 128KB, host
    L2-normalize in fp64.
  - HAM warm-up spam bridges until fnat0 lands so the convs run at the warm
    2.4 GHz PE clock (cold convs delay relu3, which stalls the in-order ACT
    queue mid-cube-stream).  fnat/ftq chunk DMAs interleave per sample.

Sharding: pure data parallel, batch 32 -> 8 cores x 4 samples.
"""

import numpy as np
import ml_dtypes

B, C, H, W, M = 32, 2048, 24, 12, 8
NCORES = 8
BL = B // NCORES          # 4 samples per core
HW = H * W                # 288
GHW = BL * HW             # 1152 packed hw across samples
P = 128
CI = C // P               # 16 conv chunks of 128 channels
NK = GHW // P             # 9 packed pool chunks
NQ = 4                    # output quadrants (512-col blocks)
CB = C // NQ              # 512
BN_EPS = 1e-3
N_SPAM = 62               # HAM warm-up matmuls
SP1 = 960                 # DVE / ACT cube split point

# transpose groups: chunks whose a3g columns are complete after sample b
TGROUPS = [(0, (0, 1)), (1, (2, 3)), (2, (4, 5)), (3, (6, 7, 8))]

_CACHE = {}


def _build_program():
    import concourse.tile as tile
    import concourse.mybir as mybir
    import concourse.bacc as bacc_mod

    # Pin every ACT function to the one table set that contains all of
    # Square/Relu/Ln/Exp, so the whole kernel does a single ACT_TABLE_LOAD.
    _orig_tables = bacc_mod.get_activation_tables

    def _pinned_tables(arch):
        tabs = dict(_orig_tables(arch))
        if "natural_log_exp_and_others" in tabs:
            for k in tabs:
                if k != "natural_log_exp_and_others":
                    tabs[k] = set()
        return tabs

    bacc_mod.get_activation_tables = _pinned_tables
    try:
        nc = _build_inner(bacc_mod, tile, mybir)
    finally:
        bacc_mod.get_activation_tables = _orig_tables
    return nc


def _build_inner(bacc, tile, mybir):
    dt = mybir.dt
    AF = mybir.ActivationFunctionType

    nc = bacc.Bacc("TRN2", target_bir_lowering=False, debug=False,
                   num_devices=NCORES)

    w2_d = nc.declare_dram_parameter("w2", [P, CI * 32 + 32], dt.float16,
                                     isOutput=False)
    dvec_d = nc.declare_dram_parameter("dvec", [32, BL], dt.float32,
                                       isOutput=False)
    fnat_d = nc.declare_dram_parameter("fnat", [BL, P, CI * HW], dt.float8e3,
                                       isOutput=False)
    ftq_d = nc.declare_dram_parameter("ftq", [P, NK * C], dt.float8e3,
                                      isOutput=False)
    out_d = nc.declare_dram_parameter("out", [P, CB], dt.float16,
                                      isOutput=True)

    with tile.TileContext(nc) as tc:
        with (
            tc.tile_pool(name="const", bufs=1) as cpool,
            tc.tile_pool(name="perst", bufs=1) as perst,
            tc.tile_pool(name="psa", bufs=2, space="PSUM") as psa_pool,
            tc.tile_pool(name="psp", bufs=1, space="PSUM") as psp_pool,
            tc.tile_pool(name="pst", bufs=2, space="PSUM") as pst_pool,
            tc.tile_pool(name="pss", bufs=1, space="PSUM") as pss_pool,
        ):
            wid = cpool.tile([P, CI * 32 + 32], dt.float16)
            w2 = wid[:, 0:CI * 32].rearrange("p (i m) -> p i m", i=CI)
            ident = wid[0:32, CI * 32:CI * 32 + 32]
            dvec = cpool.tile([32, BL], dt.float32)
            spamw = cpool.tile([P, 64], dt.float16)
            dwarm = cpool.tile([P, 128], dt.float8e3)

            fnat = [perst.tile([P, CI, HW], dt.float8e3, name=f"fnat{b}",
                               tag=f"fnat{b}") for b in range(BL)]
            ftq = [perst.tile([P, C], dt.float8e3, name=f"ftq{k}",
                              tag=f"ftq{k}") for k in range(NK)]
            f3 = [perst.tile([P, C], dt.float16, name=f"f3_{k}",
                             tag=f"f3_{k}") for k in range(NK)]
            attn = perst.tile([32, GHW], dt.float16)
            sqa = perst.tile([32, GHW], dt.float16)
            a3g = perst.tile([32, GHW], dt.float16)
            a3t = perst.tile([P, NK, 32], dt.float16)
            lnb = perst.tile([P, CB], dt.float32)
            zt = perst.tile([P, CB], dt.float16)

            psP = psp_pool.tile([P, CB], dt.float32)
            psS = pss_pool.tile([16, 64], dt.float32)

            # ---- param DMAs (scalar queue; land first) ----
            nc.scalar.dma_start(wid[:], w2_d.ap())
            nc.scalar.dma_start(dvec[:], dvec_d.ap())

            # ---- spam source (gpsimd; before spam) ----
            nc.gpsimd.memset(spamw[:], 1.0)

            # ---- feature DMAs, one HWDGE stream (sync queue) ----
            def dma_fnat(b):
                nc.sync.dma_start(fnat[b][:], fnat_d.ap()[b].rearrange(
                    "p (i hw) -> p i hw", i=CI))

            def dma_ftq(k):
                nc.sync.dma_start(ftq[k][:], ftq_d.ap()[:, C * k:C * (k + 1)])

            nc.sync.dma_start(dwarm[:], ftq_d.ap()[:, 0:128])
            dma_fnat(0)
            dma_ftq(0)
            dma_ftq(1)
            dma_fnat(1)
            dma_ftq(2)
            dma_fnat(2)
            dma_ftq(3)
            dma_fnat(3)
            for k in range(4, NK):
                dma_ftq(k)

            # ---- HAM warm-up spam (PE otherwise idle until fnat0 lands) ----
            for _ in range(N_SPAM):
                nc.tensor.matmul(psS[:, :], spamw[:, 0:16], spamw[:, 0:64],
                                 start=True, stop=True, skip_group_check=True)

            # ---- emission helpers (per-engine streams are in-order) ----
            def conv(b):
                psA = psa_pool.tile([32, CB], dt.float32, name="psA",
                                    tag="psA")
                for i in range(CI):
                    nc.tensor.matmul(
                        psA[0:32, 0:HW],
                        w2[:, i, :],
                        fnat[b][:, i, :],
                        start=(i == 0), stop=(i == CI - 1),
                        skip_group_check=True,
                    )
                return psA

            def attn_relu(b, psA):
                # psA rows 8b'+m all hold conv_b duplicates; the per-sample
                # bias column has -1e9 in the non-b bands so relu writes
                # exact zeros there, preserving a3g's block-diag structure.
                cs = slice(HW * b, HW * (b + 1))
                nc.scalar.activation(attn[0:32, cs], psA[0:32, 0:HW], AF.Relu,
                                     bias=dvec[:, b:b + 1])

            def attn_cube(b):
                cs = slice(HW * b, HW * (b + 1))
                nc.vector.tensor_mul(sqa[0:32, cs], attn[0:32, cs],
                                     attn[0:32, cs])
                nc.vector.tensor_mul(a3g[0:32, cs], sqa[0:32, cs],
                                     attn[0:32, cs])

            def transp_group(g):
                # a3t[p, k, j] = a3g[j, 128k + p] via PE transposes into one
                # grouped psum tile, then a single DVE copy for the group.
                _, ks = TGROUPS[g]
                psT = pst_pool.tile([P, 3, 32], dt.float16, name="psT",
                                    tag="psT")
                for idx, k in enumerate(ks):
                    nc.tensor.transpose(psT[:, idx, :],
                                        a3g[0:32, P * k:P * (k + 1)], ident)
                k0 = ks[0]
                n = len(ks)
                nc.vector.tensor_copy(a3t[:, k0:k0 + n, :], psT[:, 0:n, :])

            def cube_a(k):
                # DVE squares the low part of ftq -> f3 (pool operand)
                nc.vector.tensor_mul(f3[k][:, 0:SP1], ftq[k][:, 0:SP1],
                                     ftq[k][:, 0:SP1])

            def cube_b(k):
                # ACT squares the high part concurrently
                nc.scalar.activation(f3[k][:, SP1:C], ftq[k][:, SP1:C],
                                     AF.Square)

            def pool(k):
                for q in range(NQ):
                    nc.tensor.matmul(
                        psP[32 * q:32 * q + 32, :],
                        a3t[:, k, :],
                        f3[k][:, CB * q:CB * (q + 1)],
                        start=(k == 0), stop=(k == NK - 1),
                        tile_position=(0, 32 * q),
                        skip_group_check=True,
                    )

            # ---- emission (ordered by expected readiness per engine;
            # relus/attn-cubes sit behind already-fed squares so the
            # in-order ACT/DVE queues never stall on conv completion) ----
            psA0 = conv(0)
            cube_b(0)
            cube_a(0)
            psA1 = conv(1)
            cube_b(1)
            cube_a(1)
            attn_relu(0, psA0)
            psA2 = conv(2)
            cube_b(2)
            cube_a(2)
            attn_cube(0)
            transp_group(0)
            attn_relu(1, psA1)
            psA3 = conv(3)
            cube_b(3)
            cube_a(3)
            attn_cube(1)
            pool(0)
            cube_b(4)
            cube_a(4)
            attn_relu(2, psA2)
            transp_group(1)
            pool(1)
            cube_b(5)
            cube_a(5)
            attn_cube(2)
            transp_group(2)
            attn_relu(3, psA3)
            pool(2)
            pool(3)
            cube_b(6)
            cube_a(6)
            attn_cube(3)
            transp_group(3)
            pool(4)
            pool(5)
            pool(6)
            cube_b(7)
            cube_a(7)
            pool(7)
            cube_b(8)
            cube_a(8)
            pool(8)

            # ---- z = s^(1/6) -> fp16; L2 normalize happens on host ----
            # exp and the output DMA run in column halves on two queues so
            # the DRAM write overlaps the second exp
            nc.scalar.activation(lnb[:], psP[:], AF.Ln)
            nc.scalar.activation(zt[:, 0:CB // 2], lnb[:, 0:CB // 2],
                                 AF.Exp, scale=1.0 / 6.0)
            nc.sync.dma_start(out_d.ap()[:, 0:CB // 2], zt[:, 0:CB // 2])
            nc.scalar.activation(zt[:, CB // 2:CB], lnb[:, CB // 2:CB],
                                 AF.Exp, scale=1.0 / 6.0)
            nc.scalar.dma_start(out_d.ap()[:, CB // 2:CB],
                                zt[:, CB // 2:CB])

    nc.compile()
    return nc


def _host_prep(conv_w, bn_scale, bn_bias, bn_mean, bn_var):
    f16 = np.float16
    g = (bn_scale / np.sqrt(bn_var + BN_EPS)).astype(np.float32)
    d = (bn_bias - bn_mean * g).astype(np.float32)
    w2 = conv_w.astype(np.float32) * g[:, None]            # [M, C]
    # w2rep[p, i, 8b+m] = w2[m, 128i + p]  (same weights in all 4 col slots)
    w2t = w2.T.reshape(CI, P, M).transpose(1, 0, 2)        # [p, i, m]
    w2rep = np.zeros((P, CI, 32), np.float32)
    for b in range(BL):
        w2rep[:, :, 8 * b:8 * b + 8] = w2t
    wid = np.zeros((P, CI * 32 + 32), np.float32)
    wid[:, 0:CI * 32] = w2rep.reshape(P, CI * 32)
    wid[0:32, CI * 32:] = np.eye(32, dtype=np.float32)
    # dvec[:, b]: BN bias d in band 8b..8b+8, -1e9 elsewhere (relu mask)
    dvec = np.full((32, BL), -1e9, np.float32)
    for b in range(BL):
        dvec[8 * b:8 * b + 8, b] = d
    return wid.astype(f16), dvec


def _make_in_maps(features, conv_w, bn_scale, bn_bias, bn_mean, bn_var):
    e3 = ml_dtypes.float8_e3m4
    wid, dvec = _host_prep(
        np.asarray(conv_w, np.float32), np.asarray(bn_scale, np.float32),
        np.asarray(bn_bias, np.float32), np.asarray(bn_mean, np.float32),
        np.asarray(bn_var, np.float32))

    feats = np.ascontiguousarray(
        np.asarray(features, np.float32)).reshape(B, C, HW)
    in_maps = []
    for ci in range(NCORES):
        fs = feats[BL * ci:BL * (ci + 1)]                  # [BL, C, HW]
        # fnat[b][p, i*HW + h] = f[b, 128i + p, h]   (fp8 e3m4, signed)
        fnat = np.ascontiguousarray(
            np.clip(fs, -15.0, 15.0).reshape(BL, CI, P, HW)
            .transpose(0, 2, 1, 3)).reshape(BL, P, CI * HW).astype(e3)
        # ftq[p, k*C + c] = relu(f)^1.5 [G // HW, c, G % HW],  G = 128k + p
        fr = np.maximum(fs, 0.0)                           # [BL, C, HW]
        fr = fr * np.sqrt(fr)                              # relu(f)^1.5
        X = fr.transpose(0, 2, 1).reshape(GHW, C)          # [G, c]
        ftq = np.ascontiguousarray(
            X.reshape(NK, P, C).transpose(1, 0, 2)).reshape(P, NK * C)
        in_maps.append({
            "fnat": fnat, "ftq": ftq.astype(e3),
            "w2": wid, "dvec": dvec,
        })
    return in_maps


def _extract_out(arr):
    # device z rows 32q + 8b + m, cols c' -> value for c = 512q + c'.
    # Returns host-L2-normalized [BL, M*C].
    a = np.asarray(arr, np.float64).reshape(NQ, BL, M, CB)
    fm = a.transpose(1, 2, 0, 3).reshape(BL, M * C)
    n = np.linalg.norm(fm, axis=-1, keepdims=True)
    return fm / np.maximum(n, 1e-12)


def kernel(features, conv_w, bn_scale, bn_bias, bn_mean, bn_var, **_kw):
    from concourse.bass_utils import run_bass_kernel_spmd

    if "nc" not in _CACHE:
        _CACHE["nc"] = _build_program()
    nc = _CACHE["nc"]

    in_maps = _make_in_maps(features, conv_w, bn_scale, bn_bias,
                            bn_mean, bn_var)
    res = run_bass_kernel_spmd(nc, in_maps, core_ids=list(range(NCORES)),
                               **_CACHE.get("run_kwargs", {}))
    _CACHE["last_results"] = res
    out = np.concatenate(
        [_extract_out(res.results[i]["out"]) for i in range(NCORES)], axis=0)
    return np.ascontiguousarray(out.reshape(B, M * C, 1, 1).astype(np.float32))


# revision 44
# speedup vs baseline: 1.0253x; 1.0253x over previous
"""Trainium2 Bass kernel for nn_BilinearAttnPool (B=32, C=2048, H=24, W=12, M=8).

Math (exactness argument):
  reference: attn = relu(BN(conv1x1(f)))  (attn >= 0)
             x = clip(f * attn, min=1e-6) ** 3 ; pooled = mean_hw(x) ** (1/3)
  Since attn >= 0:  clip(f*attn, eps)^3 = attn^3 * relu(f)^3  up to eps^3=1e-18
  terms (negligible).  So pooled(b,m,c)^3 ~ sum_hw attn^3(b,m,hw) relu(f)^3(c,hw)
  -- a matmul over hw.  The 1/HW mean and any global scale cancel in the final
  L2 normalize, which (with the sign-sqrt; pooled >= 0 => sqrt) is done on the
  host from the device's z = s^(1/6) output.

Design (v10):
  - Dual feature upload, both fp8 e3m4 (4-bit mantissa): fnat (c-on-partitions,
    conv contraction operand, signed f) and ftq = relu(f)^1.5 (hw-on-
    partitions).  The ^1.5 encoding is range compression: e3m4's normal range
    covers [0.016, 15.5] and x^1.5 <= 12.9, so the on-device square (the
    pooling operand relu(f)^3) sees only ~2x the quantization error --
    uploading x^3 directly would overflow/underflow e3m4 and fail the gate.
    End-to-end rel-err ~9.8e-3 vs the 2e-2 gate.
  - The 4 samples' hw axes are CONCATENATED: 4*288 = 1152 = 9 exact chunks of
    128 partitions (zero padding waste).  Squares split DVE/ACT per chunk.
  - conv lhsT w2rep has the (BN-folded) weights replicated in all four 8-col
    slots, so conv(b) writes rows j=8b+m (the other bands hold junk
    duplicates).  The attn relu reads all 32 rows with a per-sample bias
    column that is -1e9 outside band b: junk rows become exact zeros, giving
    attn/a3 a block-diagonal [32, 1152] "global hw" layout.
  - 9 PE transposes (128-col slabs of a3) build a3t[p, k, 8b+m]; one matmul
    per (chunk, 512-col block) then pools all 4 samples at once (cross-sample
    terms get zero weight).  The 4 col-blocks run concurrently in the PE's
    col-tiled 32x32 sub-arrays.
  - Pool output: ONE psum bank [128, 512]; quadrant q rows 32q+8b+m hold
    c in [512q, 512q+512).  z = exp(ln(s)/6) -> fp16, DMA out 128KB, host
    L2-normalize in fp64.
  - HAM warm-up spam bridges until fnat0 lands so the convs run at the warm
    2.4 GHz PE clock (cold convs delay relu3, which stalls the in-order ACT
    queue mid-cube-stream).  fnat/ftq chunk DMAs interleave per sample.

Sharding: pure data parallel, batch 32 -> 8 cores x 4 samples.
"""

import numpy as np
import ml_dtypes

B, C, H, W, M = 32, 2048, 24, 12, 8
NCORES = 8
BL = B // NCORES          # 4 samples per core
HW = H * W                # 288
GHW = BL * HW             # 1152 packed hw across samples
P = 128
CI = C // P               # 16 conv chunks of 128 channels
NK = GHW // P             # 9 packed pool chunks
NQ = 4                    # output quadrants (512-col blocks)
CB = C // NQ              # 512
BN_EPS = 1e-3
N_SPAM = 62               # HAM warm-up matmuls
SP1 = 960                 # DVE / ACT cube split point

# transpose groups: chunks whose a3g columns are complete after sample b
TGROUPS = [(0, (0, 1)), (1, (2, 3)), (2, (4, 5)), (3, (6, 7, 8))]

_CACHE = {}


def _build_program():
    import concourse.tile as tile
    import concourse.mybir as mybir
    import concourse.bacc as bacc_mod

    # Pin every ACT function to the one table set that contains all of
    # Square/Relu/Ln/Exp, so the whole kernel does a single ACT_TABLE_LOAD.
    _orig_tables = bacc_mod.get_activation_tables

    def _pinned_tables(arch):
        tabs = dict(_orig_tables(arch))
        if "natural_log_exp_and_others" in tabs:
            for k in tabs:
                if k != "natural_log_exp_and_others":
                    tabs[k] = set()
        return tabs

    bacc_mod.get_activation_tables = _pinned_tables
    try:
        nc = _build_inner(bacc_mod, tile, mybir)
    finally:
        bacc_mod.get_activation_tables = _orig_tables
    return nc


def _build_inner(bacc, tile, mybir):
    dt = mybir.dt
    AF = mybir.ActivationFunctionType

    nc = bacc.Bacc("TRN2", target_bir_lowering=False, debug=False,
                   num_devices=NCORES)

    w2_d = nc.declare_dram_parameter("w2", [P, CI * 32 + 32], dt.float16,
                                     isOutput=False)
    dvec_d = nc.declare_dram_parameter("dvec", [32, BL], dt.float32,
                                       isOutput=False)
    fnat_d = nc.declare_dram_parameter("fnat", [BL, P, CI * HW], dt.float8e3,
                                       isOutput=False)
    ftq_d = nc.declare_dram_parameter("ftq", [P, NK * C], dt.float8e3,
                                      isOutput=False)
    out_d = nc.declare_dram_parameter("out", [P, CB], dt.float16,
                                      isOutput=True)

    with tile.TileContext(nc) as tc:
        with (
            tc.tile_pool(name="const", bufs=1) as cpool,
            tc.tile_pool(name="perst", bufs=1) as perst,
            tc.tile_pool(name="psa", bufs=2, space="PSUM") as psa_pool,
            tc.tile_pool(name="psp", bufs=1, space="PSUM") as psp_pool,
            tc.tile_pool(name="pst", bufs=2, space="PSUM") as pst_pool,
            tc.tile_pool(name="pss", bufs=1, space="PSUM") as pss_pool,
        ):
            wid = cpool.tile([P, CI * 32 + 32], dt.float16)
            w2 = wid[:, 0:CI * 32].rearrange("p (i m) -> p i m", i=CI)
            ident = wid[0:32, CI * 32:CI * 32 + 32]
            dvec = cpool.tile([32, BL], dt.float32)
            spamw = cpool.tile([P, 64], dt.float16)
            dwarm = cpool.tile([P, 128], dt.float8e3)

            fnat = [perst.tile([P, CI, HW], dt.float8e3, name=f"fnat{b}",
                               tag=f"fnat{b}") for b in range(BL)]
            ftq = [perst.tile([P, C], dt.float8e3, name=f"ftq{k}",
                              tag=f"ftq{k}") for k in range(NK)]
            f3 = [perst.tile([P, C], dt.float16, name=f"f3_{k}",
                             tag=f"f3_{k}") for k in range(NK)]
            attn = perst.tile([32, GHW], dt.float16)
            sqa = perst.tile([32, GHW], dt.float16)
            a3g = perst.tile([32, GHW], dt.float16)
            a3t = perst.tile([P, NK, 32], dt.float16)
            lnb = perst.tile([P, CB], dt.float32)
            zt = perst.tile([P, CB], dt.float16)

            psP = psp_pool.tile([P, CB], dt.float32)
            psS = pss_pool.tile([16, 64], dt.float32)

            # ---- param DMAs (scalar queue; land first) ----
            nc.scalar.dma_start(wid[:], w2_d.ap())
            nc.scalar.dma_start(dvec[:], dvec_d.ap())

            # ---- spam source (gpsimd; before spam) ----
            nc.gpsimd.memset(spamw[:], 1.0)

            # ---- feature DMAs, one HWDGE stream (sync queue) ----
            def dma_fnat(b):
                nc.sync.dma_start(fnat[b][:], fnat_d.ap()[b].rearrange(
                    "p (i hw) -> p i hw", i=CI))

            def dma_ftq(k):
                nc.sync.dma_start(ftq[k][:], ftq_d.ap()[:, C * k:C * (k + 1)])

            nc.sync.dma_start(dwarm[:], ftq_d.ap()[:, 0:128])
            dma_fnat(0)
            dma_ftq(0)
            dma_ftq(1)
            dma_fnat(1)
            dma_ftq(2)
            dma_fnat(2)
            dma_ftq(3)
            dma_fnat(3)
            for k in range(4, NK):
                dma_ftq(k)

            # ---- HAM warm-up spam (PE otherwise idle until fnat0 lands) ----
            for _ in range(N_SPAM):
                nc.tensor.matmul(psS[:, :], spamw[:, 0:16], spamw[:, 0:64],
                                 start=True, stop=True, skip_group_check=True)

            # ---- emission helpers (per-engine streams are in-order) ----
            def conv(b):
                psA = psa_pool.tile([32, CB], dt.float32, name="psA",
                                    tag="psA")
                for i in range(CI):
                    nc.tensor.matmul(
                        psA[0:32, 0:HW],
                        w2[:, i, :],
                        fnat[b][:, i, :],
                        start=(i == 0), stop=(i == CI - 1),
                        skip_group_check=True,
                    )
                return psA

            def attn_relu(b, psA):
                # psA rows 8b'+m all hold conv_b duplicates; the per-sample
                # bias column has -1e9 in the non-b bands so relu writes
                # exact zeros there, preserving a3g's block-diag structure.
                cs = slice(HW * b, HW * (b + 1))
                nc.scalar.activation(attn[0:32, cs], psA[0:32, 0:HW], AF.Relu,
                                     bias=dvec[:, b:b + 1])

            def attn_cube(b):
                cs = slice(HW * b, HW * (b + 1))
                nc.vector.tensor_mul(sqa[0:32, cs], attn[0:32, cs],
                                     attn[0:32, cs])
                nc.vector.tensor_mul(a3g[0:32, cs], sqa[0:32, cs],
                                     attn[0:32, cs])

            def transp_group(g):
                # a3t[p, k, j] = a3g[j, 128k + p] via PE transposes into one
                # grouped psum tile, then a single DVE copy for the group.
                _, ks = TGROUPS[g]
                psT = pst_pool.tile([P, 3, 32], dt.float16, name="psT",
                                    tag="psT")
                for idx, k in enumerate(ks):
                    nc.tensor.transpose(psT[:, idx, :],
                                        a3g[0:32, P * k:P * (k + 1)], ident)
                k0 = ks[0]
                n = len(ks)
                nc.vector.tensor_copy(a3t[:, k0:k0 + n, :], psT[:, 0:n, :])

            def cube_a(k):
                # DVE squares the low part of ftq -> f3 (pool operand)
                nc.vector.tensor_mul(f3[k][:, 0:SP1], ftq[k][:, 0:SP1],
                                     ftq[k][:, 0:SP1])

            def cube_b(k):
                # ACT squares the high part concurrently
                nc.scalar.activation(f3[k][:, SP1:C], ftq[k][:, SP1:C],
                                     AF.Square)

            def pool(k):
                for q in range(NQ):
                    nc.tensor.matmul(
                        psP[32 * q:32 * q + 32, :],
                        a3t[:, k, :],
                        f3[k][:, CB * q:CB * (q + 1)],
                        start=(k == 0), stop=(k == NK - 1),
                        tile_position=(0, 32 * q),
                        skip_group_check=True,
                    )

            # ---- emission (ordered by expected readiness per engine) ----
            psA0 = conv(0)
            cube_b(0)
            cube_a(0)
            attn_relu(0, psA0)
            attn_cube(0)

            psA1 = conv(1)
            cube_b(1)
            cube_a(1)
            attn_relu(1, psA1)
            transp_group(0)
            attn_cube(1)
            pool(0)

            psA2 = conv(2)
            cube_b(2)
            cube_a(2)
            attn_relu(2, psA2)
            transp_group(1)
            attn_cube(2)
            pool(1)

            psA3 = conv(3)
            cube_b(3)
            cube_a(3)
            attn_relu(3, psA3)
            transp_group(2)
            attn_cube(3)
            pool(2)
            cube_b(4)
            cube_a(4)
            pool(3)
            cube_b(5)
            cube_a(5)
            transp_group(3)
            pool(4)
            pool(5)
            cube_b(6)
            cube_a(6)
            pool(6)
            cube_b(7)
            cube_a(7)
            pool(7)
            cube_b(8)
            cube_a(8)
            pool(8)

            # ---- z = s^(1/6) -> fp16; L2 normalize happens on host ----
            # exp and the output DMA run in column halves on two queues so
            # the DRAM write overlaps the second exp
            nc.scalar.activation(lnb[:], psP[:], AF.Ln)
            nc.scalar.activation(zt[:, 0:CB // 2], lnb[:, 0:CB // 2],
                                 AF.Exp, scale=1.0 / 6.0)
            nc.sync.dma_start(out_d.ap()[:, 0:CB // 2], zt[:, 0:CB // 2])
            nc.scalar.activation(zt[:, CB // 2:CB], lnb[:, CB // 2:CB],
                                 AF.Exp, scale=1.0 / 6.0)
            nc.scalar.dma_start(out_d.ap()[:, CB // 2:CB],
                                zt[:, CB // 2:CB])

    nc.compile()
    return nc


def _host_prep(conv_w, bn_scale, bn_bias, bn_mean, bn_var):
    f16 = np.float16
    g = (bn_scale / np.sqrt(bn_var + BN_EPS)).astype(np.float32)
    d = (bn_bias - bn_mean * g).astype(np.float32)
    w2 = conv_w.astype(np.float32) * g[:, None]            # [M, C]
    # w2rep[p, i, 8b+m] = w2[m, 128i + p]  (same weights in all 4 col slots)
    w2t = w2.T.reshape(CI, P, M).transpose(1, 0, 2)        # [p, i, m]
    w2rep = np.zeros((P, CI, 32), np.float32)
    for b in range(BL):
        w2rep[:, :, 8 * b:8 * b + 8] = w2t
    wid = np.zeros((P, CI * 32 + 32), np.float32)
    wid[:, 0:CI * 32] = w2rep.reshape(P, CI * 32)
    wid[0:32, CI * 32:] = np.eye(32, dtype=np.float32)
    # dvec[:, b]: BN bias d in band 8b..8b+8, -1e9 elsewhere (relu mask)
    dvec = np.full((32, BL), -1e9, np.float32)
    for b in range(BL):
        dvec[8 * b:8 * b + 8, b] = d
    return wid.astype(f16), dvec


def _make_in_maps(features, conv_w, bn_scale, bn_bias, bn_mean, bn_var):
    e3 = ml_dtypes.float8_e3m4
    wid, dvec = _host_prep(
        np.asarray(conv_w, np.float32), np.asarray(bn_scale, np.float32),
        np.asarray(bn_bias, np.float32), np.asarray(bn_mean, np.float32),
        np.asarray(bn_var, np.float32))

    feats = np.ascontiguousarray(
        np.asarray(features, np.float32)).reshape(B, C, HW)
    in_maps = []
    for ci in range(NCORES):
        fs = feats[BL * ci:BL * (ci + 1)]                  # [BL, C, HW]
        # fnat[b][p, i*HW + h] = f[b, 128i + p, h]   (fp8 e3m4, signed)
        fnat = np.ascontiguousarray(
            np.clip(fs, -15.0, 15.0).reshape(BL, CI, P, HW)
            .transpose(0, 2, 1, 3)).reshape(BL, P, CI * HW).astype(e3)
        # ftq[p, k*C + c] = relu(f)^1.5 [G // HW, c, G % HW],  G = 128k + p
        fr = np.maximum(fs, 0.0)                           # [BL, C, HW]
        fr = fr * np.sqrt(fr)                              # relu(f)^1.5
        X = fr.transpose(0, 2, 1).reshape(GHW, C)          # [G, c]
        ftq = np.ascontiguousarray(
            X.reshape(NK, P, C).transpose(1, 0, 2)).reshape(P, NK * C)
        in_maps.append({
            "fnat": fnat, "ftq": ftq.astype(e3),
            "w2": wid, "dvec": dvec,
        })
    return in_maps


def _extract_out(arr):
    # device z rows 32q + 8b + m, cols c' -> value for c = 512q + c'.
    # Returns host-L2-normalized [BL, M*C].
    a = np.asarray(arr, np.float64).reshape(NQ, BL, M, CB)
    fm = a.transpose(1, 2, 0, 3).reshape(BL, M * C)
    n = np.linalg.norm(fm, axis=-1, keepdims=True)
    return fm / np.maximum(n, 1e-12)


def kernel(features, conv_w, bn_scale, bn_bias, bn_mean, bn_var, **_kw):
    from concourse.bass_utils import run_bass_kernel_spmd

    if "nc" not in _CACHE:
        _CACHE["nc"] = _build_program()
    nc = _CACHE["nc"]

    in_maps = _make_in_maps(features, conv_w, bn_scale, bn_bias,
                            bn_mean, bn_var)
    res = run_bass_kernel_spmd(nc, in_maps, core_ids=list(range(NCORES)),
                               **_CACHE.get("run_kwargs", {}))
    _CACHE["last_results"] = res
    out = np.concatenate(
        [_extract_out(res.results[i]["out"]) for i in range(NCORES)], axis=0)
    return np.ascontiguousarray(out.reshape(B, M * C, 1, 1).astype(np.float32))


# revision 45
# speedup vs baseline: 1.0373x; 1.0116x over previous
"""Trainium2 Bass kernel for nn_BilinearAttnPool (B=32, C=2048, H=24, W=12, M=8).

Math (exactness argument):
  reference: attn = relu(BN(conv1x1(f)))  (attn >= 0)
             x = clip(f * attn, min=1e-6) ** 3 ; pooled = mean_hw(x) ** (1/3)
  Since attn >= 0:  clip(f*attn, eps)^3 = attn^3 * relu(f)^3  up to eps^3=1e-18
  terms (negligible).  So pooled(b,m,c)^3 ~ sum_hw attn^3(b,m,hw) relu(f)^3(c,hw)
  -- a matmul over hw.  The 1/HW mean and any global scale cancel in the final
  L2 normalize, which (with the sign-sqrt; pooled >= 0 => sqrt) is done on the
  host from the device's z = s^(1/6) output.

Design (v10):
  - Dual feature upload, both fp8 e3m4 (4-bit mantissa): fnat (c-on-partitions,
    conv contraction operand, signed f) and ftq = relu(f)^1.5 (hw-on-
    partitions).  The ^1.5 encoding is range compression: e3m4's normal range
    covers [0.016, 15.5] and x^1.5 <= 12.9, so the on-device square (the
    pooling operand relu(f)^3) sees only ~2x the quantization error --
    uploading x^3 directly would overflow/underflow e3m4 and fail the gate.
    End-to-end rel-err ~9.8e-3 vs the 2e-2 gate.
  - The 4 samples' hw axes are CONCATENATED: 4*288 = 1152 = 9 exact chunks of
    128 partitions (zero padding waste).  Squares split DVE/ACT per chunk.
  - conv lhsT w2rep has the (BN-folded) weights replicated in all four 8-col
    slots, so conv(b) writes rows j=8b+m (the other bands hold junk
    duplicates).  The attn relu reads all 32 rows with a per-sample bias
    column that is -1e9 outside band b: junk rows become exact zeros, giving
    attn/a3 a block-diagonal [32, 1152] "global hw" layout.
  - 9 PE transposes (128-col slabs of a3) build a3t[p, k, 8b+m]; one matmul
    per (chunk, 512-col block) then pools all 4 samples at once (cross-sample
    terms get zero weight).  The 4 col-blocks run concurrently in the PE's
    col-tiled 32x32 sub-arrays.
  - Pool output: ONE psum bank [128, 512]; quadrant q rows 32q+8b+m hold
    c in [512q, 512q+512).  z = exp(ln(s)/6) -> fp16, DMA out 128KB, host
    L2-normalize in fp64.
  - HAM warm-up spam bridges until fnat0 lands so the convs run at the warm
    2.4 GHz PE clock (cold convs delay relu3, which stalls the in-order ACT
    queue mid-cube-stream).  fnat/ftq chunk DMAs interleave per sample.

Sharding: pure data parallel, batch 32 -> 8 cores x 4 samples.
"""

import numpy as np
import ml_dtypes

B, C, H, W, M = 32, 2048, 24, 12, 8
NCORES = 8
BL = B // NCORES          # 4 samples per core
HW = H * W                # 288
GHW = BL * HW             # 1152 packed hw across samples
P = 128
CI = C // P               # 16 conv chunks of 128 channels
NK = GHW // P             # 9 packed pool chunks
NQ = 4                    # output quadrants (512-col blocks)
CB = C // NQ              # 512
BN_EPS = 1e-3
N_SPAM = 62               # HAM warm-up matmuls
SP1 = 960                 # DVE / ACT cube split point

# transpose groups: chunks whose a3g columns are complete after sample b
TGROUPS = [(0, (0, 1)), (1, (2, 3)), (2, (4, 5)), (3, (6, 7, 8))]

_CACHE = {}


def _build_program():
    import concourse.tile as tile
    import concourse.mybir as mybir
    import concourse.bacc as bacc_mod

    # Pin every ACT function to the one table set that contains all of
    # Square/Relu/Ln/Exp, so the whole kernel does a single ACT_TABLE_LOAD.
    _orig_tables = bacc_mod.get_activation_tables

    def _pinned_tables(arch):
        tabs = dict(_orig_tables(arch))
        if "natural_log_exp_and_others" in tabs:
            for k in tabs:
                if k != "natural_log_exp_and_others":
                    tabs[k] = set()
        return tabs

    bacc_mod.get_activation_tables = _pinned_tables
    try:
        nc = _build_inner(bacc_mod, tile, mybir)
    finally:
        bacc_mod.get_activation_tables = _orig_tables
    return nc


def _build_inner(bacc, tile, mybir):
    dt = mybir.dt
    AF = mybir.ActivationFunctionType

    nc = bacc.Bacc("TRN2", target_bir_lowering=False, debug=False,
                   num_devices=NCORES)

    w2_d = nc.declare_dram_parameter("w2", [P, CI * 32 + 32], dt.float16,
                                     isOutput=False)
    dvec_d = nc.declare_dram_parameter("dvec", [32, BL], dt.float32,
                                       isOutput=False)
    fnat_d = nc.declare_dram_parameter("fnat", [BL, P, CI * HW], dt.float8e3,
                                       isOutput=False)
    ftq_d = nc.declare_dram_parameter("ftq", [P, NK * C], dt.float8e3,
                                      isOutput=False)
    out_d = nc.declare_dram_parameter("out", [P, CB], dt.float16,
                                      isOutput=True)

    with tile.TileContext(nc) as tc:
        with (
            tc.tile_pool(name="const", bufs=1) as cpool,
            tc.tile_pool(name="perst", bufs=1) as perst,
            tc.tile_pool(name="psa", bufs=3, space="PSUM") as psa_pool,
            tc.tile_pool(name="psp", bufs=1, space="PSUM") as psp_pool,
            tc.tile_pool(name="pst", bufs=3, space="PSUM") as pst_pool,
            tc.tile_pool(name="pss", bufs=1, space="PSUM") as pss_pool,
        ):
            wid = cpool.tile([P, CI * 32 + 32], dt.float16)
            w2 = wid[:, 0:CI * 32].rearrange("p (i m) -> p i m", i=CI)
            ident = wid[0:32, CI * 32:CI * 32 + 32]
            dvec = cpool.tile([32, BL], dt.float32)
            spamw = cpool.tile([P, 64], dt.float16)
            dwarm = cpool.tile([P, 128], dt.float8e3)

            fnat = [perst.tile([P, CI, HW], dt.float8e3, name=f"fnat{b}",
                               tag=f"fnat{b}") for b in range(BL)]
            ftq = [perst.tile([P, C], dt.float8e3, name=f"ftq{k}",
                              tag=f"ftq{k}") for k in range(NK)]
            f3 = [perst.tile([P, C], dt.float16, name=f"f3_{k}",
                             tag=f"f3_{k}") for k in range(NK)]
            attn = perst.tile([32, GHW], dt.float16)
            sqa = perst.tile([32, GHW], dt.float16)
            a3g = perst.tile([32, GHW], dt.float16)
            a3t = perst.tile([P, NK, 32], dt.float16)
            lnb = perst.tile([P, CB], dt.float32)
            zt = perst.tile([P, CB], dt.float16)

            psP = psp_pool.tile([P, CB], dt.float32)
            psS = pss_pool.tile([16, 64], dt.float32)

            # ---- param DMAs (scalar queue; land first) ----
            nc.scalar.dma_start(wid[:], w2_d.ap())
            nc.scalar.dma_start(dvec[:], dvec_d.ap())

            # ---- spam source (gpsimd; before spam) ----
            nc.gpsimd.memset(spamw[:], 1.0)

            # ---- feature DMAs, one HWDGE stream (sync queue) ----
            def dma_fnat(b):
                nc.sync.dma_start(fnat[b][:], fnat_d.ap()[b].rearrange(
                    "p (i hw) -> p i hw", i=CI))

            def dma_ftq(k):
                nc.sync.dma_start(ftq[k][:], ftq_d.ap()[:, C * k:C * (k + 1)])

            nc.sync.dma_start(dwarm[:], ftq_d.ap()[:, 0:128])
            dma_fnat(0)
            dma_ftq(0)
            dma_ftq(1)
            dma_fnat(1)
            dma_ftq(2)
            dma_fnat(2)
            dma_ftq(3)
            dma_fnat(3)
            for k in range(4, NK):
                dma_ftq(k)

            # ---- HAM warm-up spam (PE otherwise idle until fnat0 lands) ----
            for _ in range(N_SPAM):
                nc.tensor.matmul(psS[:, :], spamw[:, 0:16], spamw[:, 0:64],
                                 start=True, stop=True, skip_group_check=True)

            # ---- emission helpers (per-engine streams are in-order) ----
            def conv(b):
                psA = psa_pool.tile([32, CB], dt.float32, name="psA",
                                    tag="psA")
                for i in range(CI):
                    nc.tensor.matmul(
                        psA[0:32, 0:HW],
                        w2[:, i, :],
                        fnat[b][:, i, :],
                        start=(i == 0), stop=(i == CI - 1),
                        skip_group_check=True,
                    )
                return psA

            def attn_relu(b, psA):
                # psA rows 8b'+m all hold conv_b duplicates; the per-sample
                # bias column has -1e9 in the non-b bands so relu writes
                # exact zeros there, preserving a3g's block-diag structure.
                cs = slice(HW * b, HW * (b + 1))
                nc.scalar.activation(attn[0:32, cs], psA[0:32, 0:HW], AF.Relu,
                                     bias=dvec[:, b:b + 1])

            def attn_cube(b):
                cs = slice(HW * b, HW * (b + 1))
                nc.vector.tensor_mul(sqa[0:32, cs], attn[0:32, cs],
                                     attn[0:32, cs])
                nc.vector.tensor_mul(a3g[0:32, cs], sqa[0:32, cs],
                                     attn[0:32, cs])

            def transp_group(g):
                # a3t[p, k, j] = a3g[j, 128k + p] via PE transposes into one
                # grouped psum tile, then a single DVE copy for the group.
                _, ks = TGROUPS[g]
                psT = pst_pool.tile([P, 3, 32], dt.float16, name="psT",
                                    tag="psT")
                for idx, k in enumerate(ks):
                    nc.tensor.transpose(psT[:, idx, :],
                                        a3g[0:32, P * k:P * (k + 1)], ident)
                k0 = ks[0]
                n = len(ks)
                nc.vector.tensor_copy(a3t[:, k0:k0 + n, :], psT[:, 0:n, :])

            def cube_a(k):
                # DVE squares the low part of ftq -> f3 (pool operand)
                nc.vector.tensor_mul(f3[k][:, 0:SP1], ftq[k][:, 0:SP1],
                                     ftq[k][:, 0:SP1])

            def cube_b(k):
                # ACT squares the high part concurrently
                nc.scalar.activation(f3[k][:, SP1:C], ftq[k][:, SP1:C],
                                     AF.Square)

            def pool(k):
                for q in range(NQ):
                    nc.tensor.matmul(
                        psP[32 * q:32 * q + 32, :],
                        a3t[:, k, :],
                        f3[k][:, CB * q:CB * (q + 1)],
                        start=(k == 0), stop=(k == NK - 1),
                        tile_position=(0, 32 * q),
                        skip_group_check=True,
                    )

            # ---- emission (ordered by expected readiness per engine) ----
            psA0 = conv(0)
            cube_b(0)
            cube_a(0)
            attn_relu(0, psA0)
            attn_cube(0)

            psA1 = conv(1)
            cube_b(1)
            cube_a(1)
            attn_relu(1, psA1)
            transp_group(0)
            attn_cube(1)
            pool(0)

            psA2 = conv(2)
            cube_b(2)
            cube_a(2)
            attn_relu(2, psA2)
            transp_group(1)
            attn_cube(2)
            pool(1)

            psA3 = conv(3)
            cube_b(3)
            cube_a(3)
            attn_relu(3, psA3)
            transp_group(2)
            attn_cube(3)
            pool(2)
            cube_b(4)
            cube_a(4)
            pool(3)
            cube_b(5)
            cube_a(5)
            transp_group(3)
            pool(4)
            pool(5)
            cube_b(6)
            cube_a(6)
            pool(6)
            cube_b(7)
            cube_a(7)
            pool(7)
            cube_b(8)
            cube_a(8)
            pool(8)

            # ---- z = s^(1/6) -> fp16; L2 normalize happens on host ----
            # exp and the output DMA run in column halves on two queues so
            # the DRAM write overlaps the second exp
            nc.scalar.activation(lnb[:], psP[:], AF.Ln)
            nc.scalar.activation(zt[:, 0:CB // 2], lnb[:, 0:CB // 2],
                                 AF.Exp, scale=1.0 / 6.0)
            nc.sync.dma_start(out_d.ap()[:, 0:CB // 2], zt[:, 0:CB // 2])
            nc.scalar.activation(zt[:, CB // 2:CB], lnb[:, CB // 2:CB],
                                 AF.Exp, scale=1.0 / 6.0)
            nc.scalar.dma_start(out_d.ap()[:, CB // 2:CB],
                                zt[:, CB // 2:CB])

    nc.compile()
    return nc


def _host_prep(conv_w, bn_scale, bn_bias, bn_mean, bn_var):
    f16 = np.float16
    g = (bn_scale / np.sqrt(bn_var + BN_EPS)).astype(np.float32)
    d = (bn_bias - bn_mean * g).astype(np.float32)
    w2 = conv_w.astype(np.float32) * g[:, None]            # [M, C]
    # w2rep[p, i, 8b+m] = w2[m, 128i + p]  (same weights in all 4 col slots)
    w2t = w2.T.reshape(CI, P, M).transpose(1, 0, 2)        # [p, i, m]
    w2rep = np.zeros((P, CI, 32), np.float32)
    for b in range(BL):
        w2rep[:, :, 8 * b:8 * b + 8] = w2t
    wid = np.zeros((P, CI * 32 + 32), np.float32)
    wid[:, 0:CI * 32] = w2rep.reshape(P, CI * 32)
    wid[0:32, CI * 32:] = np.eye(32, dtype=np.float32)
    # dvec[:, b]: BN bias d in band 8b..8b+8, -1e9 elsewhere (relu mask)
    dvec = np.full((32, BL), -1e9, np.float32)
    for b in range(BL):
        dvec[8 * b:8 * b + 8, b] = d
    return wid.astype(f16), dvec


def _make_in_maps(features, conv_w, bn_scale, bn_bias, bn_mean, bn_var):
    e3 = ml_dtypes.float8_e3m4
    wid, dvec = _host_prep(
        np.asarray(conv_w, np.float32), np.asarray(bn_scale, np.float32),
        np.asarray(bn_bias, np.float32), np.asarray(bn_mean, np.float32),
        np.asarray(bn_var, np.float32))

    feats = np.ascontiguousarray(
        np.asarray(features, np.float32)).reshape(B, C, HW)
    in_maps = []
    for ci in range(NCORES):
        fs = feats[BL * ci:BL * (ci + 1)]                  # [BL, C, HW]
        # fnat[b][p, i*HW + h] = f[b, 128i + p, h]   (fp8 e3m4, signed)
        fnat = np.ascontiguousarray(
            np.clip(fs, -15.0, 15.0).reshape(BL, CI, P, HW)
            .transpose(0, 2, 1, 3)).reshape(BL, P, CI * HW).astype(e3)
        # ftq[p, k*C + c] = relu(f)^1.5 [G // HW, c, G % HW],  G = 128k + p
        fr = np.maximum(fs, 0.0)                           # [BL, C, HW]
        fr = fr * np.sqrt(fr)                              # relu(f)^1.5
        X = fr.transpose(0, 2, 1).reshape(GHW, C)          # [G, c]
        ftq = np.ascontiguousarray(
            X.reshape(NK, P, C).transpose(1, 0, 2)).reshape(P, NK * C)
        in_maps.append({
            "fnat": fnat, "ftq": ftq.astype(e3),
            "w2": wid, "dvec": dvec,
        })
    return in_maps


def _extract_out(arr):
    # device z rows 32q + 8b + m, cols c' -> value for c = 512q + c'.
    # Returns host-L2-normalized [BL, M*C].
    a = np.asarray(arr, np.float64).reshape(NQ, BL, M, CB)
    fm = a.transpose(1, 2, 0, 3).reshape(BL, M * C)
    n = np.linalg.norm(fm, axis=-1, keepdims=True)
    return fm / np.maximum(n, 1e-12)


def kernel(features, conv_w, bn_scale, bn_bias, bn_mean, bn_var, **_kw):
    from concourse.bass_utils import run_bass_kernel_spmd

    if "nc" not in _CACHE:
        _CACHE["nc"] = _build_program()
    nc = _CACHE["nc"]

    in_maps = _make_in_maps(features, conv_w, bn_scale, bn_bias,
                            bn_mean, bn_var)
    res = run_bass_kernel_spmd(nc, in_maps, core_ids=list(range(NCORES)),
                               **_CACHE.get("run_kwargs", {}))
    _CACHE["last_results"] = res
    out = np.concatenate(
        [_extract_out(res.results[i]["out"]) for i in range(NCORES)], axis=0)
    return np.ascontiguousarray(out.reshape(B, M * C, 1, 1).astype(np.float32))


# revision 46
# speedup vs baseline: 1.0786x; 1.0399x over previous
"""Trainium2 Bass kernel for nn_BilinearAttnPool (B=32, C=2048, H=24, W=12, M=8).

Math (exactness argument):
  reference: attn = relu(BN(conv1x1(f)))  (attn >= 0)
             x = clip(f * attn, min=1e-6) ** 3 ; pooled = mean_hw(x) ** (1/3)
  Since attn >= 0:  clip(f*attn, eps)^3 = attn^3 * relu(f)^3  up to eps^3=1e-18
  terms (negligible).  So pooled(b,m,c)^3 ~ sum_hw attn^3(b,m,hw) relu(f)^3(c,hw)
  -- a matmul over hw.  The 1/HW mean and any global scale cancel in the final
  L2 normalize, which (with the sign-sqrt; pooled >= 0 => sqrt) is done on the
  host from the device's z = s^(1/6) output.

Design (v10):
  - Dual feature upload, both fp8 e3m4 (4-bit mantissa): fnat (c-on-partitions,
    conv contraction operand, signed f) and ftq = relu(f)^1.5 (hw-on-
    partitions).  The ^1.5 encoding is range compression: e3m4's normal range
    covers [0.016, 15.5] and x^1.5 <= 12.9, so the on-device square (the
    pooling operand relu(f)^3) sees only ~2x the quantization error --
    uploading x^3 directly would overflow/underflow e3m4 and fail the gate.
    End-to-end rel-err ~9.8e-3 vs the 2e-2 gate.
  - The 4 samples' hw axes are CONCATENATED: 4*288 = 1152 = 9 exact chunks of
    128 partitions (zero padding waste).  Squares split DVE/ACT per chunk.
  - conv lhsT w2rep has the (BN-folded) weights replicated in all four 8-col
    slots, so conv(b) writes rows j=8b+m (the other bands hold junk
    duplicates).  The attn relu reads all 32 rows with a per-sample bias
    column that is -1e9 outside band b: junk rows become exact zeros, giving
    attn/a3 a block-diagonal [32, 1152] "global hw" layout.
  - 9 PE transposes (128-col slabs of a3) build a3t[p, k, 8b+m]; one matmul
    per (chunk, 512-col block) then pools all 4 samples at once (cross-sample
    terms get zero weight).  The 4 col-blocks run concurrently in the PE's
    col-tiled 32x32 sub-arrays.
  - Pool output: ONE psum bank [128, 512]; quadrant q rows 32q+8b+m hold
    c in [512q, 512q+512).  z = exp(ln(s)/6) -> fp16, DMA out 128KB, host
    L2-normalize in fp64.
  - HAM warm-up spam bridges until fnat0 lands so the convs run at the warm
    2.4 GHz PE clock (cold convs delay relu3, which stalls the in-order ACT
    queue mid-cube-stream).  fnat/ftq chunk DMAs interleave per sample.

Sharding: pure data parallel, batch 32 -> 8 cores x 4 samples.
"""

import numpy as np
import ml_dtypes

B, C, H, W, M = 32, 2048, 24, 12, 8
NCORES = 8
BL = B // NCORES          # 4 samples per core
HW = H * W                # 288
GHW = BL * HW             # 1152 packed hw across samples
P = 128
CI = C // P               # 16 conv chunks of 128 channels
NK = GHW // P             # 9 packed pool chunks
NQ = 4                    # output quadrants (512-col blocks)
CB = C // NQ              # 512
BN_EPS = 1e-3
N_SPAM = 26               # HAM warm-up matmuls (FD=256, ~97% duty)
SP1 = 960                 # DVE / ACT cube split point

# transpose groups: chunks whose a3g columns are complete after sample b
TGROUPS = [(0, (0, 1)), (1, (2, 3)), (2, (4, 5)), (3, (6, 7, 8))]

_CACHE = {}


def _build_program():
    import concourse.tile as tile
    import concourse.mybir as mybir
    import concourse.bacc as bacc_mod

    # Pin every ACT function to the one table set that contains all of
    # Square/Relu/Ln/Exp, so the whole kernel does a single ACT_TABLE_LOAD.
    _orig_tables = bacc_mod.get_activation_tables

    def _pinned_tables(arch):
        tabs = dict(_orig_tables(arch))
        if "natural_log_exp_and_others" in tabs:
            for k in tabs:
                if k != "natural_log_exp_and_others":
                    tabs[k] = set()
        return tabs

    bacc_mod.get_activation_tables = _pinned_tables
    try:
        nc = _build_inner(bacc_mod, tile, mybir)
    finally:
        bacc_mod.get_activation_tables = _orig_tables
    return nc


def _build_inner(bacc, tile, mybir):
    dt = mybir.dt
    AF = mybir.ActivationFunctionType

    nc = bacc.Bacc("TRN2", target_bir_lowering=False, debug=False,
                   num_devices=NCORES)

    w2_d = nc.declare_dram_parameter("w2", [P, CI * 32 + 32], dt.float16,
                                     isOutput=False)
    dvec_d = nc.declare_dram_parameter("dvec", [32, BL], dt.float32,
                                       isOutput=False)
    fnat_d = nc.declare_dram_parameter("fnat", [BL, P, CI * HW], dt.float8e3,
                                       isOutput=False)
    ftq_d = nc.declare_dram_parameter("ftq", [P, NK * C], dt.float8e3,
                                      isOutput=False)
    out_d = nc.declare_dram_parameter("out", [P, CB], dt.float16,
                                      isOutput=True)

    with tile.TileContext(nc) as tc:
        with (
            tc.tile_pool(name="const", bufs=1) as cpool,
            tc.tile_pool(name="perst", bufs=1) as perst,
            tc.tile_pool(name="psa", bufs=3, space="PSUM") as psa_pool,
            tc.tile_pool(name="psp", bufs=1, space="PSUM") as psp_pool,
            tc.tile_pool(name="pst", bufs=3, space="PSUM") as pst_pool,
            tc.tile_pool(name="pss", bufs=1, space="PSUM") as pss_pool,
        ):
            wid = cpool.tile([P, CI * 32 + 32], dt.float16)
            w2 = wid[:, 0:CI * 32].rearrange("p (i m) -> p i m", i=CI)
            ident = wid[0:32, CI * 32:CI * 32 + 32]
            dvec = cpool.tile([32, BL], dt.float32)
            spamw = cpool.tile([P, 256], dt.float16)
            dwarm = cpool.tile([P, 128], dt.float8e3)

            fnat = [perst.tile([P, CI, HW], dt.float8e3, name=f"fnat{b}",
                               tag=f"fnat{b}") for b in range(BL)]
            ftq = [perst.tile([P, C], dt.float8e3, name=f"ftq{k}",
                              tag=f"ftq{k}") for k in range(NK)]
            f3 = [perst.tile([P, C], dt.float16, name=f"f3_{k}",
                             tag=f"f3_{k}") for k in range(NK)]
            attn = perst.tile([32, GHW], dt.float16)
            sqa = perst.tile([32, GHW], dt.float16)
            a3g = perst.tile([32, GHW], dt.float16)
            a3t = perst.tile([P, NK, 32], dt.float16)
            lnb = perst.tile([P, CB], dt.float32)
            zt = perst.tile([P, CB], dt.float16)

            psP = psp_pool.tile([P, CB], dt.float32)
            psS = pss_pool.tile([16, 256], dt.float32)

            # ---- param DMAs (scalar queue; land first) ----
            nc.scalar.dma_start(wid[:], w2_d.ap())
            nc.scalar.dma_start(dvec[:], dvec_d.ap())

            # ---- spam source (gpsimd; before spam) ----
            nc.gpsimd.memset(spamw[:], 1.0)

            # ---- feature DMAs, one HWDGE stream (sync queue) ----
            def dma_fnat(b):
                nc.sync.dma_start(fnat[b][:], fnat_d.ap()[b].rearrange(
                    "p (i hw) -> p i hw", i=CI))

            def dma_ftq(k):
                nc.sync.dma_start(ftq[k][:], ftq_d.ap()[:, C * k:C * (k + 1)])

            nc.sync.dma_start(dwarm[:], ftq_d.ap()[:, 0:128])
            dma_fnat(0)
            dma_ftq(0)
            dma_ftq(1)
            dma_fnat(1)
            dma_ftq(2)
            dma_fnat(2)
            dma_ftq(3)
            dma_fnat(3)
            for k in range(4, NK):
                dma_ftq(k)

            # ---- HAM warm-up spam (PE otherwise idle until fnat0 lands) ----
            for _ in range(N_SPAM):
                nc.tensor.matmul(psS[:, :], spamw[:, 0:16], spamw[:, :],
                                 start=True, stop=True, skip_group_check=True)

            # ---- emission helpers (per-engine streams are in-order) ----
            def conv(b):
                psA = psa_pool.tile([32, CB], dt.float32, name="psA",
                                    tag="psA")
                for i in range(CI):
                    nc.tensor.matmul(
                        psA[0:32, 0:HW],
                        w2[:, i, :],
                        fnat[b][:, i, :],
                        start=(i == 0), stop=(i == CI - 1),
                        skip_group_check=True,
                    )
                return psA

            def attn_relu(b, psA):
                # psA rows 8b'+m all hold conv_b duplicates; the per-sample
                # bias column has -1e9 in the non-b bands so relu writes
                # exact zeros there, preserving a3g's block-diag structure.
                cs = slice(HW * b, HW * (b + 1))
                nc.scalar.activation(attn[0:32, cs], psA[0:32, 0:HW], AF.Relu,
                                     bias=dvec[:, b:b + 1])

            def attn_cube(b):
                cs = slice(HW * b, HW * (b + 1))
                nc.vector.tensor_mul(sqa[0:32, cs], attn[0:32, cs],
                                     attn[0:32, cs])
                nc.vector.tensor_mul(a3g[0:32, cs], sqa[0:32, cs],
                                     attn[0:32, cs])

            def transp_group(g):
                # a3t[p, k, j] = a3g[j, 128k + p] via PE transposes into one
                # grouped psum tile, then a single DVE copy for the group.
                _, ks = TGROUPS[g]
                psT = pst_pool.tile([P, 3, 32], dt.float16, name="psT",
                                    tag="psT")
                for idx, k in enumerate(ks):
                    nc.tensor.transpose(psT[:, idx, :],
                                        a3g[0:32, P * k:P * (k + 1)], ident)
                k0 = ks[0]
                n = len(ks)
                nc.vector.tensor_copy(a3t[:, k0:k0 + n, :], psT[:, 0:n, :])

            def cube_a(k):
                # DVE squares the low part of ftq -> f3 (pool operand)
                nc.vector.tensor_mul(f3[k][:, 0:SP1], ftq[k][:, 0:SP1],
                                     ftq[k][:, 0:SP1])

            def cube_b(k):
                # ACT squares the high part concurrently
                nc.scalar.activation(f3[k][:, SP1:C], ftq[k][:, SP1:C],
                                     AF.Square)

            def pool(k):
                for q in range(NQ):
                    nc.tensor.matmul(
                        psP[32 * q:32 * q + 32, :],
                        a3t[:, k, :],
                        f3[k][:, CB * q:CB * (q + 1)],
                        start=(k == 0), stop=(k == NK - 1),
                        tile_position=(0, 32 * q),
                        skip_group_check=True,
                    )

            # ---- emission (ordered by expected readiness per engine) ----
            psA0 = conv(0)
            cube_b(0)
            cube_a(0)
            attn_relu(0, psA0)
            attn_cube(0)

            psA1 = conv(1)
            cube_b(1)
            cube_a(1)
            attn_relu(1, psA1)
            transp_group(0)
            attn_cube(1)
            pool(0)

            psA2 = conv(2)
            cube_b(2)
            cube_a(2)
            attn_relu(2, psA2)
            transp_group(1)
            attn_cube(2)
            pool(1)

            psA3 = conv(3)
            cube_b(3)
            cube_a(3)
            attn_relu(3, psA3)
            transp_group(2)
            attn_cube(3)
            pool(2)
            cube_b(4)
            cube_a(4)
            pool(3)
            cube_b(5)
            cube_a(5)
            transp_group(3)
            pool(4)
            pool(5)
            cube_b(6)
            cube_a(6)
            pool(6)
            cube_b(7)
            cube_a(7)
            pool(7)
            cube_b(8)
            cube_a(8)
            pool(8)

            # ---- z = s^(1/6) -> fp16; L2 normalize happens on host ----
            # exp and the output DMA run in column halves on two queues so
            # the DRAM write overlaps the second exp
            nc.scalar.activation(lnb[:], psP[:], AF.Ln)
            nc.scalar.activation(zt[:, 0:CB // 2], lnb[:, 0:CB // 2],
                                 AF.Exp, scale=1.0 / 6.0)
            nc.sync.dma_start(out_d.ap()[:, 0:CB // 2], zt[:, 0:CB // 2])
            nc.scalar.activation(zt[:, CB // 2:CB], lnb[:, CB // 2:CB],
                                 AF.Exp, scale=1.0 / 6.0)
            nc.scalar.dma_start(out_d.ap()[:, CB // 2:CB],
                                zt[:, CB // 2:CB])

    nc.compile()
    return nc


def _host_prep(conv_w, bn_scale, bn_bias, bn_mean, bn_var):
    f16 = np.float16
    g = (bn_scale / np.sqrt(bn_var + BN_EPS)).astype(np.float32)
    d = (bn_bias - bn_mean * g).astype(np.float32)
    w2 = conv_w.astype(np.float32) * g[:, None]            # [M, C]
    # w2rep[p, i, 8b+m] = w2[m, 128i + p]  (same weights in all 4 col slots)
    w2t = w2.T.reshape(CI, P, M).transpose(1, 0, 2)        # [p, i, m]
    w2rep = np.zeros((P, CI, 32), np.float32)
    for b in range(BL):
        w2rep[:, :, 8 * b:8 * b + 8] = w2t
    wid = np.zeros((P, CI * 32 + 32), np.float32)
    wid[:, 0:CI * 32] = w2rep.reshape(P, CI * 32)
    wid[0:32, CI * 32:] = np.eye(32, dtype=np.float32)
    # dvec[:, b]: BN bias d in band 8b..8b+8, -1e9 elsewhere (relu mask)
    dvec = np.full((32, BL), -1e9, np.float32)
    for b in range(BL):
        dvec[8 * b:8 * b + 8, b] = d
    return wid.astype(f16), dvec


def _make_in_maps(features, conv_w, bn_scale, bn_bias, bn_mean, bn_var):
    e3 = ml_dtypes.float8_e3m4
    wid, dvec = _host_prep(
        np.asarray(conv_w, np.float32), np.asarray(bn_scale, np.float32),
        np.asarray(bn_bias, np.float32), np.asarray(bn_mean, np.float32),
        np.asarray(bn_var, np.float32))

    feats = np.ascontiguousarray(
        np.asarray(features, np.float32)).reshape(B, C, HW)
    in_maps = []
    for ci in range(NCORES):
        fs = feats[BL * ci:BL * (ci + 1)]                  # [BL, C, HW]
        # fnat[b][p, i*HW + h] = f[b, 128i + p, h]   (fp8 e3m4, signed)
        fnat = np.ascontiguousarray(
            np.clip(fs, -15.0, 15.0).reshape(BL, CI, P, HW)
            .transpose(0, 2, 1, 3)).reshape(BL, P, CI * HW).astype(e3)
        # ftq[p, k*C + c] = relu(f)^1.5 [G // HW, c, G % HW],  G = 128k + p
        fr = np.maximum(fs, 0.0)                           # [BL, C, HW]
        fr = fr * np.sqrt(fr)                              # relu(f)^1.5
        X = fr.transpose(0, 2, 1).reshape(GHW, C)          # [G, c]
        ftq = np.ascontiguousarray(
            X.reshape(NK, P, C).transpose(1, 0, 2)).reshape(P, NK * C)
        in_maps.append({
            "fnat": fnat, "ftq": ftq.astype(e3),
            "w2": wid, "dvec": dvec,
        })
    return in_maps


def _extract_out(arr):
    # device z rows 32q + 8b + m, cols c' -> value for c = 512q + c'.
    # Returns host-L2-normalized [BL, M*C].
    a = np.asarray(arr, np.float64).reshape(NQ, BL, M, CB)
    fm = a.transpose(1, 2, 0, 3).reshape(BL, M * C)
    n = np.linalg.norm(fm, axis=-1, keepdims=True)
    return fm / np.maximum(n, 1e-12)


def kernel(features, conv_w, bn_scale, bn_bias, bn_mean, bn_var, **_kw):
    from concourse.bass_utils import run_bass_kernel_spmd

    if "nc" not in _CACHE:
        _CACHE["nc"] = _build_program()
    nc = _CACHE["nc"]

    in_maps = _make_in_maps(features, conv_w, bn_scale, bn_bias,
                            bn_mean, bn_var)
    res = run_bass_kernel_spmd(nc, in_maps, core_ids=list(range(NCORES)),
                               **_CACHE.get("run_kwargs", {}))
    _CACHE["last_results"] = res
    out = np.concatenate(
        [_extract_out(res.results[i]["out"]) for i in range(NCORES)], axis=0)
    return np.ascontiguousarray(out.reshape(B, M * C, 1, 1).astype(np.float32))
